# revision 1
# baseline (speedup 1.0000x reference)
"""Bit2Num dequantization kernel for Trainium2 (8 NeuronCores, SPMD).

Reference op: x [1024, 65536] of {0.0, 1.0} f32, B=4.
  bits = x.reshape(1024, 16384, 4)
  out[b, n] = (8*bits[b,n,0] + 4*bits[b,n,1] + 2*bits[b,n,2] + bits[b,n,3] + 0.5) / 16

Sharding: pure data-parallel over batch — 128 rows per core (= 128 SBUF
partitions). Per core: 32 MB in + 8 MB out => DMA-roofline-bound (~117 us
at ~358 GB/s HBM-per-NC).

Per-core kernel: pipeline over 8 column tiles of [128, 8192], computed in
half-tile chunks (quarters on the last tile to shrink the exposed tail):
  DMA-in tile (nc.sync ring) -> per chunk: 3 fused scalar_tensor_tensor ops
  on DVE (u=2a+b, v=2c+d, w=4u+v over the 4 strided bit slices) -> final
  affine (w/16 + 1/32) on ACT -> DMA-out on the nc.scalar ring (separate
  HWDGE ring, so stores never stall the in-stream).
Loads are SWDGE (gpsimd) DMAs casting f32->bf16 in-flight: SBUF-side
write traffic halves (the ~435 GB/s SDMA fabric is shared with stores),
leaving the 32 MB HBM read at ~356 GB/s as the binding stream. Tapered
trailing segments (4x1 MB) keep the final compute/store tail small.
Deep work/out pools (bufs=4) keep DVE's w-slot and ACT's ot-slot
recycling off the critical path at the stream tail.
Measured: bit-exact; clean-core DMA window ~108 us, span ~120 us.
"""

import numpy as np

import concourse.bacc as bacc
import concourse.bass as bass
import concourse.mybir as mybir
from concourse.bass_utils import run_bass_kernel_spmd
from concourse.tile import TileContext

N_CORES = 8
BATCH = 1024
COLS = 65536
B_BITS = 4
ROWS = BATCH // N_CORES          # 128 rows per core == SBUF partition count
OUT_COLS = COLS // B_BITS        # 16384
TILE_C = 8192                    # input cols per tile (32 KB / partition)
TILE_G = TILE_C // B_BITS        # 2048 output cols per tile
N_TILES = COLS // TILE_C         # 8

F32 = mybir.dt.float32
BF16 = mybir.dt.bfloat16
MULT = mybir.AluOpType.mult
ADD = mybir.AluOpType.add


def _build_nc() -> bass.Bass:
    # Bacc (not plain Bass): its compile() pipeline runs
    # generate_event_semaphores, which splits multi-wait sync conditions —
    # TRN2 DMA instructions accept at most one wait.
    nc = bacc.Bacc(None, target_bir_lowering=False)
    x = nc.dram_tensor("x", [ROWS, COLS], F32, kind="ExternalInput")
    out = nc.dram_tensor("out", [ROWS, OUT_COLS], F32, kind="ExternalOutput")

    # Segment list (in-DMA sizes + per-segment compute chunks). The stream
    # tapers at the end: each trailing 1 MB in-DMA gates only one small
    # chunk, so nearly all compute/stores overlap the in-stream instead of
    # queueing behind the final 4 MB transfer.
    segments = [(4096, [1024])] * 14 + [(2048, [512])] * 4
    assert sum(s[0] for s in segments) == COLS

    with TileContext(nc) as tc:
        with (
            # 2 MB bf16 segments: DVE starts each segment's chunk ~6 us
            # after issue instead of ~12, halving end-of-stream phase lag.
            tc.tile_pool(name="xin", bufs=8) as xpool,
            tc.tile_pool(name="work", bufs=4) as wpool,
            tc.tile_pool(name="oout", bufs=4) as opool,
        ):
            col = 0
            g_off = 0
            for seg_c, chunk_gs in segments:
                xt = xpool.tile([ROWS, seg_c], BF16, tag="xt")
                # SWDGE in-DMAs with f32 -> bf16 cast: halves the SBUF-side
                # write traffic, which shares the ~435 GB/s SDMA fabric with
                # the stores. 0.0/1.0 are exact in bf16.
                nc.gpsimd.dma_start(
                    out=xt[:, :], in_=x[:, col:col + seg_c]
                )
                col += seg_c
                c_off = 0
                for chunk_g in chunk_gs:
                    chunk_c = chunk_g * B_BITS
                    xv = xt[:, c_off:c_off + chunk_c].rearrange(
                        "p (g k) -> p g k", k=B_BITS
                    )
                    c_off += chunk_c
                    a = xv[:, :, 0]
                    b = xv[:, :, 1]
                    c = xv[:, :, 2]
                    d = xv[:, :, 3]

                    # intermediates stay bf16 (all values <= 15, exact);
                    # ACT casts back to f32 on the final affine.
                    u = wpool.tile([ROWS, chunk_g], BF16, tag="u")
                    v = wpool.tile([ROWS, chunk_g], BF16, tag="v")
                    w = wpool.tile([ROWS, chunk_g], BF16, tag="w")
                    ot = opool.tile([ROWS, chunk_g], F32, tag="ot")

                    # u = 2a + b ; v = 2c + d ; w = 4u + v = 8a+4b+2c+d
                    nc.vector.scalar_tensor_tensor(
                        out=u[:, :], in0=a, scalar=2.0, in1=b,
                        op0=MULT, op1=ADD,
                    )
                    nc.vector.scalar_tensor_tensor(
                        out=v[:, :], in0=c, scalar=2.0, in1=d,
                        op0=MULT, op1=ADD,
                    )
                    nc.vector.scalar_tensor_tensor(
                        out=w[:, :], in0=u[:, :], scalar=4.0, in1=v[:, :],
                        op0=MULT, op1=ADD,
                    )
                    # ot = (w + 0.5) / 16 = w/16 + 1/32
                    nc.scalar.activation(
                        out=ot[:, :], in_=w[:, :],
                        func=mybir.ActivationFunctionType.Copy,
                        bias=1.0 / 32.0, scale=1.0 / 16.0,
                    )
                    # out-DMAs on the ACT HWDGE ring (qActDynamicHW) so a
                    # store waiting on compute never blocks the in-stream.
                    nc.scalar.dma_start(
                        out=out[:, g_off:g_off + chunk_g], in_=ot[:, :]
                    )
                    g_off += chunk_g
    # Bacc.finalize runs the compile pipeline (register allocation +
    # generate_event_semaphores); the pjrt exec path serializes nc.m as-is.
    nc.finalize()
    return nc


_NC = None


def _get_nc() -> bass.Bass:
    global _NC
    if _NC is None:
        _NC = _build_nc()
    return _NC


def kernel(x: np.ndarray, B=4) -> np.ndarray:
    assert int(B) == B_BITS, f"kernel hardcodes B={B_BITS}, got {B}"
    x = np.ascontiguousarray(x, dtype=np.float32)
    assert x.shape == (BATCH, COLS), x.shape
    nc = _get_nc()
    in_maps = [{"x": x[i * ROWS:(i + 1) * ROWS]} for i in range(N_CORES)]
    res = run_bass_kernel_spmd(nc, in_maps, list(range(N_CORES)))
    return np.concatenate(
        [res.results[i]["out"] for i in range(N_CORES)], axis=0
    )



# revision 2
# speedup vs baseline: 1.0664x; 1.0664x over previous
"""Bit2Num dequantization kernel for Trainium2 (8 NeuronCores, SPMD).

Reference op: x [1024, 65536] of {0.0, 1.0} f32, B=4.
  bits = x.reshape(1024, 16384, 4)
  out[b, n] = (8*bits[b,n,0] + 4*bits[b,n,1] + 2*bits[b,n,2] + bits[b,n,3] + 0.5) / 16

Sharding: pure data-parallel over batch — 128 rows per core (= 128 SBUF
partitions). Per core: 32 MB f32 in + 16K outputs/row.

The stream is SDMA-engine-aggregate bound (~450 GB/s/core counting the
larger side of each transfer), so total engine-side bytes are what
matter. Per-core engine traffic:
  in : 32 MB (f32 HBM read, cast f32->bf16 in-flight on the SWDGE path —
       SBUF-side traffic halves; 0.0/1.0 are exact in bf16)
  out: 4 MB  (output staged as bf16 in DRAM — every output value is
       (2k+1)/32, k<16, needing <=5 significand bits, exact in bf16;
       the host widens back to f32 losslessly with astype)
=> ~37.8 MB at ~450 GB/s ≈ 84 us stream window.

Per-core kernel: 11 column segments (7 x 8192 cols, then a 4096/2048/
1024/1024 taper to shrink the exposed tail). Each segment is one
compute chain: 3 fused scalar_tensor_tensor ops on DVE over the 4
strided bit slices (u=2a+b, v=2c+d, w=4u+v), then the affine
(w/16 + 1/32) with the bf16 down-cast on ACT, then a store on the
scalar HWDGE ring (qActDynamicHW — independent of the SWDGE in-stream).
Few, large DMAs (11 in + 11 out) keep the Tile semaphore-reset epilogue
and Q7 descriptor-emission overheads small.
"""

import numpy as np

import concourse.bacc as bacc
import concourse.bass as bass
import concourse.mybir as mybir
from concourse.bass_utils import run_bass_kernel_spmd
from concourse.tile import TileContext

N_CORES = 8
BATCH = 1024
COLS = 65536
B_BITS = 4
ROWS = BATCH // N_CORES          # 128 rows per core == SBUF partition count
OUT_COLS = COLS // B_BITS        # 16384

F32 = mybir.dt.float32
BF16 = mybir.dt.bfloat16
MULT = mybir.AluOpType.mult
ADD = mybir.AluOpType.add

# Input-column segment sizes. Large head segments amortize descriptor
# emission; the tapered tail keeps the last exposed compute+store short.
SEGMENTS = [8192] * 7 + [4096, 2048, 1024, 1024]
assert sum(SEGMENTS) == COLS


def _build_nc() -> bass.Bass:
    # Bacc (not plain Bass): its compile() pipeline runs
    # generate_event_semaphores, which splits multi-wait sync conditions —
    # TRN2 DMA instructions accept at most one wait.
    nc = bacc.Bacc(None, target_bir_lowering=False)
    x = nc.dram_tensor("x", [ROWS, COLS], F32, kind="ExternalInput")
    out = nc.dram_tensor("out", [ROWS, OUT_COLS], BF16, kind="ExternalOutput")

    with TileContext(nc) as tc:
        with (
            tc.tile_pool(name="xin", bufs=5) as xpool,
            tc.tile_pool(name="work", bufs=3) as wpool,
            tc.tile_pool(name="oout", bufs=3) as opool,
        ):
            col = 0
            for seg_c in SEGMENTS:
                g = seg_c // B_BITS
                g0 = col // B_BITS
                xt = xpool.tile([ROWS, seg_c], BF16, tag="xt")
                # SWDGE in-DMA with f32 -> bf16 cast: HBM reads the full
                # f32 input; SBUF-side write traffic halves.
                nc.gpsimd.dma_start(out=xt[:, :], in_=x[:, col:col + seg_c])
                col += seg_c

                xv = xt[:, :].rearrange("p (g k) -> p g k", k=B_BITS)
                a = xv[:, :, 0]
                b = xv[:, :, 1]
                c = xv[:, :, 2]
                d = xv[:, :, 3]

                # intermediates stay bf16 (all values <= 15, exact)
                u = wpool.tile([ROWS, g], BF16, tag="u")
                v = wpool.tile([ROWS, g], BF16, tag="v")
                w = wpool.tile([ROWS, g], BF16, tag="w")
                ot = opool.tile([ROWS, g], BF16, tag="ot")

                # u = 2a + b ; v = 2c + d ; w = 4u + v = 8a+4b+2c+d
                nc.vector.scalar_tensor_tensor(
                    out=u[:, :], in0=a, scalar=2.0, in1=b, op0=MULT, op1=ADD,
                )
                nc.vector.scalar_tensor_tensor(
                    out=v[:, :], in0=c, scalar=2.0, in1=d, op0=MULT, op1=ADD,
                )
                nc.vector.scalar_tensor_tensor(
                    out=w[:, :], in0=u[:, :], scalar=4.0, in1=v[:, :],
                    op0=MULT, op1=ADD,
                )
                # ot = (w + 0.5) / 16 = w/16 + 1/32, down-cast to bf16
                # (exact). ACT computes the affine in fp32 internally.
                nc.scalar.activation(
                    out=ot[:, :], in_=w[:, :],
                    func=mybir.ActivationFunctionType.Copy,
                    bias=1.0 / 32.0, scale=1.0 / 16.0,
                )
                # out-DMA on the ACT HWDGE ring (qActDynamicHW) so stores
                # never contend with the SWDGE in-stream for a sequencer.
                nc.scalar.dma_start(out=out[:, g0:g0 + g], in_=ot[:, :])
    # Bacc.finalize runs the compile pipeline (register allocation +
    # generate_event_semaphores); the pjrt exec path serializes nc.m as-is.
    nc.finalize()
    return nc


_NC = None


def _get_nc() -> bass.Bass:
    global _NC
    if _NC is None:
        _NC = _build_nc()
    return _NC


def kernel(x: np.ndarray, B=4) -> np.ndarray:
    assert int(B) == B_BITS, f"kernel hardcodes B={B_BITS}, got {B}"
    x = np.ascontiguousarray(x, dtype=np.float32)
    assert x.shape == (BATCH, COLS), x.shape
    nc = _get_nc()
    in_maps = [{"x": x[i * ROWS:(i + 1) * ROWS]} for i in range(N_CORES)]
    res = run_bass_kernel_spmd(nc, in_maps, list(range(N_CORES)))
    # Device output is bf16 (exact for these values); widen losslessly.
    return np.concatenate(
        [np.asarray(res.results[i]["out"]) for i in range(N_CORES)], axis=0
    ).astype(np.float32)


# revision 3
# speedup vs baseline: 1.2443x; 1.1669x over previous
"""Bit2Num dequantization kernel for Trainium2 (8 NeuronCores, SPMD).

Reference op: x [1024, 65536] of {0.0, 1.0} f32, B=4.
  bits = x.reshape(1024, 16384, 4)
  out[b, n] = (8*bits[b,n,0] + 4*bits[b,n,1] + 2*bits[b,n,2] + bits[b,n,3] + 0.5) / 16

Sharding: pure data-parallel over batch — 128 rows per core (= 128 SBUF
partitions). Per core: 32 MB f32 in, 16K outputs/row.

The stream is SDMA-engine-aggregate bound (~435-460 GB/s/core, counting
the larger side of each transfer), so total engine-side bytes are what
matter:
  in : 32 MB f32 HBM read (irreducible — the engine cost is the f32
       side whether or not the SBUF side is cast down)
  out: 4 MB bf16 (output staged as bf16 in DRAM — every output value is
       (2k+1)/32, k<16, needing <=5 significand bits, exact in bf16;
       the host widens back to f32 losslessly with astype)
=> ~37.8 MB => ~85-90 us stream window.

Loads ride the sync HWDGE ring (qSPDynamicHW) as plain f32: SWDGE
(gpsimd) loads were measured to strand a single-engine straggler —
SDMA engine 15 runs ~18% slow under SWDGE (descriptor-ring AXI port
contention, a documented TRN2 erratum), and its accumulated backlog
serialized the last ~17 us of the stream at ~11 GB/s. HWDGE has no
SBUF descriptor ring, so all 16 engines drain evenly. Stores ride the
separate scalar HWDGE ring (qActDynamicHW).

Per-core kernel: 19 column segments (15 x 4096 cols, then a 2048/1024/
512/512 taper so the last exposed compute+store is tiny). Each segment
is one compute chain: 3 fused scalar_tensor_tensor ops on DVE over the
4 strided bit slices (u=2a+b, v=2c+d, w=4u+v, bf16 out — exact), then
the affine (w/16 + 1/32) on ACT writing the bf16 output tile.
"""

import numpy as np

import concourse.bacc as bacc
import concourse.bass as bass
import concourse.mybir as mybir
from concourse.bass_utils import run_bass_kernel_spmd
from concourse.tile import TileContext

N_CORES = 8
BATCH = 1024
COLS = 65536
B_BITS = 4
ROWS = BATCH // N_CORES          # 128 rows per core == SBUF partition count
OUT_COLS = COLS // B_BITS        # 16384

F32 = mybir.dt.float32
BF16 = mybir.dt.bfloat16
MULT = mybir.AluOpType.mult
ADD = mybir.AluOpType.add

# Input-column segment sizes. 4096-col segments (16 KB/partition f32)
# keep the load pipeline fine-grained; the tapered tail keeps the last
# exposed compute+store short.
SEGMENTS = [4096] * 15 + [2048, 1024, 512, 512]
assert sum(SEGMENTS) == COLS


def _build_nc() -> bass.Bass:
    # Bacc (not plain Bass): its compile() pipeline runs
    # generate_event_semaphores, which splits multi-wait sync conditions —
    # TRN2 DMA instructions accept at most one wait.
    nc = bacc.Bacc(None, target_bir_lowering=False)
    x = nc.dram_tensor("x", [ROWS, COLS], F32, kind="ExternalInput")
    out = nc.dram_tensor("out", [ROWS, OUT_COLS], BF16, kind="ExternalOutput")

    with TileContext(nc) as tc:
        with (
            tc.tile_pool(name="xin", bufs=6) as xpool,
            tc.tile_pool(name="work", bufs=3) as wpool,
            tc.tile_pool(name="oout", bufs=4) as opool,
        ):
            col = 0
            for seg_c in SEGMENTS:
                g = seg_c // B_BITS
                g0 = col // B_BITS
                xt = xpool.tile([ROWS, seg_c], F32, tag="xt")
                # HWDGE in-DMA (sync ring, qSPDynamicHW), plain f32.
                nc.sync.dma_start(out=xt[:, :], in_=x[:, col:col + seg_c])
                col += seg_c

                xv = xt[:, :].rearrange("p (g k) -> p g k", k=B_BITS)
                a = xv[:, :, 0]
                b = xv[:, :, 1]
                c = xv[:, :, 2]
                d = xv[:, :, 3]

                # intermediates in bf16 (all values <= 15, exact); DVE
                # computes in fp32 internally either way, and the strided
                # reads run in 1x mode at the same rate for f32 and bf16.
                u = wpool.tile([ROWS, g], BF16, tag="u")
                v = wpool.tile([ROWS, g], BF16, tag="v")
                w = wpool.tile([ROWS, g], BF16, tag="w")
                ot = opool.tile([ROWS, g], BF16, tag="ot")

                # u = 2a + b ; v = 2c + d ; w = 4u + v = 8a+4b+2c+d
                nc.vector.scalar_tensor_tensor(
                    out=u[:, :], in0=a, scalar=2.0, in1=b, op0=MULT, op1=ADD,
                )
                nc.vector.scalar_tensor_tensor(
                    out=v[:, :], in0=c, scalar=2.0, in1=d, op0=MULT, op1=ADD,
                )
                nc.vector.scalar_tensor_tensor(
                    out=w[:, :], in0=u[:, :], scalar=4.0, in1=v[:, :],
                    op0=MULT, op1=ADD,
                )
                # ot = (w + 0.5) / 16 = w/16 + 1/32, computed fp32-internal
                # on ACT, written bf16 (exact).
                nc.scalar.activation(
                    out=ot[:, :], in_=w[:, :],
                    func=mybir.ActivationFunctionType.Copy,
                    bias=1.0 / 32.0, scale=1.0 / 16.0,
                )
                # out-DMA on the ACT HWDGE ring (qActDynamicHW) so stores
                # never contend with the in-stream for a sequencer slot.
                nc.scalar.dma_start(out=out[:, g0:g0 + g], in_=ot[:, :])
    # Bacc.finalize runs the compile pipeline (register allocation +
    # generate_event_semaphores); the pjrt exec path serializes nc.m as-is.
    nc.finalize()
    return nc


_NC = None


def _get_nc() -> bass.Bass:
    global _NC
    if _NC is None:
        _NC = _build_nc()
    return _NC


def kernel(x: np.ndarray, B=4) -> np.ndarray:
    assert int(B) == B_BITS, f"kernel hardcodes B={B_BITS}, got {B}"
    x = np.ascontiguousarray(x, dtype=np.float32)
    assert x.shape == (BATCH, COLS), x.shape
    nc = _get_nc()
    in_maps = [{"x": x[i * ROWS:(i + 1) * ROWS]} for i in range(N_CORES)]
    res = run_bass_kernel_spmd(nc, in_maps, list(range(N_CORES)))
    # Device output is bf16 (exact for these values); widen losslessly.
    return np.concatenate(
        [np.asarray(res.results[i]["out"]) for i in range(N_CORES)], axis=0
    ).astype(np.float32)
